# revision 7
# baseline (speedup 1.0000x reference)
"""Trainium2 Bass kernel for nn_CategoricalDecoder (topk_masking).

Math (validated against the reference offline):
  logits = relu(z @ W1 + b1) @ W2 + b2, reshaped (N, D, C)
  logp   = log_softmax(logits, -1)
  scores_full[b,n] = sum_d logp[n, d, x[b,d]]          (all D features)
  scores_tail[b,n] = sum_{d in last 4} logp[n,d,x[b,d]] (the reference's
        (g*mask_full).sum - (g*mask_obs).sum collapses to the last-4 sum)
  top-16 bins by scores_tail; num = scores_full at those bins,
  den = num - scores_tail there; out = lse(num) - lse(den).

Device algorithm (8 cores, bin axis N sharded 1024/core, batch replicated):
  - per-class gather is a matmul with one-hot(x) rows (built on host)
  - log_softmax denominators folded into the score matmul via 2 extra
    contraction rows (coefficients -1) carrying S32[n] = sum_d log(sumexp)
    and S4[n] (last-4 sum); inner softmax needs no max-stabilization
    (|logits| < 5 for this net scale).
  - top-16 per row via DVE max(top-8) + match_replace + max, local per
    shard; AllGather the 8x16 local candidates; the global 16th largest
    is a threshold T; masked logsumexp with mask (tail >= T) needs no
    gather/argmax at all. Partial exp-sums AllReduce'd across cores.
"""

import numpy as np
from contextlib import ExitStack

import concourse.bass as bass
import concourse.bacc as bacc
import concourse.tile as tile
from concourse import mybir
from concourse.bass_utils import run_bass_kernel_spmd

F32 = mybir.dt.float32
AF = mybir.ActivationFunctionType
ALU = mybir.AluOpType

B, N, Lz, H, D, C = 256, 8192, 64, 256, 32, 16
DC = D * C          # 512
P = 8               # cores
NL = N // P         # 1024 bins per core
K = 16
NEG = -1.0e30


def _build_nc():
    nc = bacc.Bacc("TRN2", target_bir_lowering=False, num_devices=P)

    zT = nc.declare_dram_parameter("zT", [Lz, NL], F32, isOutput=False)
    w1 = nc.declare_dram_parameter("w1", [Lz, H], F32, isOutput=False)
    w2s = nc.declare_dram_parameter("w2s", [128, 2 * DC], F32, isOutput=False)
    b1s = nc.declare_dram_parameter("b1s", [128, 2], F32, isOutput=False)
    b2s = nc.declare_dram_parameter("b2s", [128, 4], F32, isOutput=False)
    ohs = nc.declare_dram_parameter("ohs", [128, 4 * 256], F32, isOutput=False)
    gsel = nc.declare_dram_parameter("gsel", [128, 4 * 32], F32, isOutput=False)
    ones_s = nc.declare_dram_parameter("ones_s", [32, 2], F32, isOutput=False)
    coef = nc.declare_dram_parameter("coef", [2, 256], F32, isOutput=False)
    outp = nc.declare_dram_parameter("out", [B], F32, isOutput=True)

    with tile.TileContext(nc) as tc, ExitStack() as ctx:
        const = ctx.enter_context(tc.tile_pool(name="const", bufs=1))
        act = ctx.enter_context(tc.tile_pool(name="act", bufs=1))
        ppool = ctx.enter_context(tc.tile_pool(name="psum", bufs=3, space="PSUM"))
        spsum = ctx.enter_context(tc.tile_pool(name="spsum", bufs=1, space="PSUM"))
        dram = ctx.enter_context(tc.tile_pool(name="dram", bufs=1, space="DRAM"))
        scratch = ctx.enter_context(tc.tile_pool(name="scratch", bufs=2))

        # ---- load params to SBUF
        zT_sb = const.tile([Lz, NL], F32)
        nc.sync.dma_start(zT_sb[:], zT[:])
        w1_sb = const.tile([Lz, H], F32)
        nc.sync.dma_start(w1_sb[:], w1[:])
        w2_sb = const.tile([128, 2 * DC], F32)
        nc.sync.dma_start(w2_sb[:], w2s[:])
        b1_sb = const.tile([128, 2], F32)
        nc.sync.dma_start(b1_sb[:], b1s[:])
        b2_sb = const.tile([128, 4], F32)
        nc.sync.dma_start(b2_sb[:], b2s[:])
        oh_sb = const.tile([128, 4 * 256], F32)
        nc.sync.dma_start(oh_sb[:], ohs[:])
        g_sb = const.tile([128, 4 * 32], F32)
        nc.sync.dma_start(g_sb[:], gsel[:])
        on_sb = const.tile([32, 2], F32)
        nc.sync.dma_start(on_sb[:], ones_s[:])
        cf_sb = const.tile([2, 256], F32)
        nc.sync.dma_start(cf_sb[:], coef[:])

        # ---- stage 1: hT[2][128, NL] = relu(W1.T @ zT + b1)
        hT = []
        for m in range(2):
            ph = ppool.tile([128, NL], F32, tag="mm")
            for f in range(NL // 512):
                nc.tensor.matmul(
                    ph[:, f * 512:(f + 1) * 512],
                    w1_sb[:, m * 128:(m + 1) * 128],
                    zT_sb[:, f * 512:(f + 1) * 512],
                    start=True, stop=True,
                )
            h = act.tile([128, NL], F32, name=f"hT{m}")
            nc.scalar.activation(h[:], ph[:], AF.Relu, bias=b1_sb[:, m:m + 1])
            hT.append(h)

        # ---- stage 2: lin[t] = logitsT + b2 (SBUF), e[t] = exp(lin) (SBUF)
        lin, ee = [], []
        for t in range(4):
            pl = ppool.tile([128, NL], F32, tag="mm")
            for f in range(NL // 512):
                for kk in range(2):
                    nc.tensor.matmul(
                        pl[:, f * 512:(f + 1) * 512],
                        w2_sb[:, kk * DC + t * 128: kk * DC + (t + 1) * 128],
                        hT[kk][:, f * 512:(f + 1) * 512],
                        start=(kk == 0), stop=(kk == 1),
                    )
            li = act.tile([128, NL], F32, name=f"lin{t}")
            nc.vector.tensor_scalar_add(li[:], pl[:], b2_sb[:, t:t + 1])
            ex = act.tile([128, NL], F32, name=f"e{t}")
            nc.scalar.activation(ex[:], pl[:], AF.Exp, bias=b2_sb[:, t:t + 1])
            lin.append(li)
            ee.append(ex)

        # ---- stage 3: group-sums of e -> L = log(sumexp) [32, NL] -> S rows [2, NL]
        pse = spsum.tile([34, NL], F32, tag="se")
        for f in range(NL // 512):
            for t in range(4):
                nc.tensor.matmul(
                    pse[0:32, f * 512:(f + 1) * 512],
                    g_sb[:, t * 32:(t + 1) * 32],
                    ee[t][:, f * 512:(f + 1) * 512],
                    start=(t == 0), stop=(t == 3),
                )
        l_sb = act.tile([32, NL], F32, name="l_sb")
        nc.scalar.activation(l_sb[:], pse[0:32, :], AF.Ln)
        for f in range(NL // 512):
            nc.tensor.matmul(
                pse[32:34, f * 512:(f + 1) * 512], on_sb[:, 0:2],
                l_sb[:, f * 512:(f + 1) * 512], start=True, stop=True,
            )
        s_sb = act.tile([2, NL], F32, name="s_sb")
        nc.vector.tensor_copy(s_sb[:], pse[32:34, :])

        # ---- stage 4: score matmuls -> sf/st [bt][128, NL] in SBUF
        sf, st, dn = [], [], []
        for bt in range(2):
            pf = ppool.tile([128, NL], F32, tag="mm")
            pt = ppool.tile([128, NL], F32, tag="mm")
            for f in range(NL // 512):
                sl = slice(f * 512, (f + 1) * 512)
                for t in range(4):
                    nc.tensor.matmul(
                        pf[:, sl],
                        oh_sb[:, t * 256 + bt * 128: t * 256 + (bt + 1) * 128],
                        lin[t][:, sl], start=(t == 0), stop=False,
                    )
                nc.tensor.matmul(
                    pf[:, sl], cf_sb[:, 0:128], s_sb[:, sl],
                    start=False, stop=True,
                )
                nc.tensor.matmul(
                    pt[:, sl],
                    oh_sb[64:128, 3 * 256 + bt * 128: 3 * 256 + (bt + 1) * 128],
                    lin[3][64:128, sl], start=True, stop=False,
                )
                nc.tensor.matmul(
                    pt[:, sl], cf_sb[:, 128:256], s_sb[:, sl],
                    start=False, stop=True,
                )
            f_sb = act.tile([128, NL], F32, name=f"sf{bt}")
            nc.scalar.copy(f_sb[:], pf[:])
            t_sb = act.tile([128, NL], F32, name=f"st{bt}")
            nc.vector.tensor_copy(t_sb[:], pt[:])
            d_sb = act.tile([128, NL], F32, name=f"dn{bt}")
            nc.vector.tensor_sub(d_sb[:], f_sb[:], t_sb[:])
            sf.append(f_sb)
            st.append(t_sb)
            dn.append(d_sb)

        # ---- stage 5: local top-16 of tail + local row maxes; exchange
        xin = dram.tile([B, 18], F32)
        xout = dram.tile([P * B, 18], F32, addr_space="Shared")
        for bt in range(2):
            x_sb = act.tile([128, 18], F32, name=f"x{bt}")
            nc.vector.max(x_sb[:, 0:8], st[bt][:])
            wk = scratch.tile([128, NL], F32, tag="wk")
            nc.vector.match_replace(wk[:], x_sb[:, 0:8], st[bt][:], NEG)
            nc.vector.max(x_sb[:, 8:16], wk[:])
            nc.vector.reduce_max(x_sb[:, 16:17], sf[bt][:], axis=mybir.AxisListType.X)
            nc.vector.reduce_max(x_sb[:, 17:18], dn[bt][:], axis=mybir.AxisListType.X)
            nc.sync.dma_start(xin[bt * 128:(bt + 1) * 128, :], x_sb[:])
        nc.gpsimd.collective_compute(
            "AllGather", ALU.bypass, replica_groups=[list(range(P))],
            ins=[xin[:].opt()], outs=[xout[:].opt()],
        )

        # ---- stage 6: global threshold + masked logsumexp partial sums
        sin = dram.tile([B, 2], F32)
        sout = dram.tile([B, 2], F32, addr_space="Shared")
        gmx = []  # [128,2] per bt: col0 = -gmax_num, col1 = -gmax_den
        tthr = []
        for bt in range(2):
            y_sb = act.tile([128, P, 18], F32, name=f"y{bt}")
            for c in range(P):
                nc.sync.dma_start(
                    y_sb[:, c, :],
                    xout[c * B + bt * 128: c * B + (bt + 1) * 128, :],
                )
            cand = act.tile([128, P * 16], F32, name=f"cand{bt}")
            nc.vector.tensor_copy(
                cand[:].rearrange("p (a b) -> p a b", a=P), y_sb[:, :, 0:16])
            g8 = act.tile([128, 16], F32, name=f"g8{bt}")
            nc.vector.max(g8[:, 0:8], cand[:])
            wk2 = scratch.tile([128, P * 16], F32, tag="wk2")
            nc.vector.match_replace(wk2[:], g8[:, 0:8], cand[:], NEG)
            nc.vector.max(g8[:, 8:16], wk2[:])
            tthr.append(g8)
            gm = act.tile([128, 2], F32, name=f"gm{bt}")
            nc.vector.tensor_reduce(
                gm[:, 0:1], y_sb[:, :, 16:17], axis=mybir.AxisListType.XY,
                op=ALU.max, negate=True)
            nc.vector.tensor_reduce(
                gm[:, 1:2], y_sb[:, :, 17:18], axis=mybir.AxisListType.XY,
                op=ALU.max, negate=True)
            gmx.append(gm)

            exn = scratch.tile([128, NL], F32, tag="exn")
            nc.scalar.activation(exn[:], sf[bt][:], AF.Exp, bias=gm[:, 0:1])
            s2 = act.tile([128, 2], F32, name=f"s2{bt}")
            msk = scratch.tile([128, NL], F32, tag="msk")
            nc.vector.scalar_tensor_tensor(
                msk[:], st[bt][:], g8[:, 15:16], exn[:],
                op0=ALU.is_ge, op1=ALU.mult, accum_out=s2[:, 0:1])
            exd = scratch.tile([128, NL], F32, tag="exn")
            nc.scalar.activation(exd[:], dn[bt][:], AF.Exp, bias=gm[:, 1:2])
            msk2 = scratch.tile([128, NL], F32, tag="msk")
            nc.vector.scalar_tensor_tensor(
                msk2[:], st[bt][:], g8[:, 15:16], exd[:],
                op0=ALU.is_ge, op1=ALU.mult, accum_out=s2[:, 1:2])
            nc.sync.dma_start(sin[bt * 128:(bt + 1) * 128, :], s2[:])

        nc.gpsimd.collective_compute(
            "AllReduce", ALU.add, replica_groups=[list(range(P))],
            ins=[sin[:].opt()], outs=[sout[:].opt()],
        )

        # ---- stage 7: out = (gmax_n + ln(sum_n)) - (gmax_d + ln(sum_d))
        for bt in range(2):
            gs = act.tile([128, 2], F32, name=f"gs{bt}")
            nc.sync.dma_start(gs[:], sout[bt * 128:(bt + 1) * 128, :])
            ln = act.tile([128, 2], F32, name=f"ln{bt}")
            nc.scalar.activation(ln[:], gs[:], AF.Ln)
            # gmx holds NEGATED maxes: out = (ln_n - ln_d) - (negmax_n - negmax_d)
            t1 = act.tile([128, 1], F32, name=f"t1{bt}")
            nc.vector.tensor_sub(t1[:], ln[:, 0:1], ln[:, 1:2])
            t2 = act.tile([128, 1], F32, name=f"t2{bt}")
            nc.vector.tensor_sub(t2[:], gmx[bt][:, 1:2], gmx[bt][:, 0:1])
            t3 = act.tile([128, 1], F32, name=f"t3{bt}")
            nc.vector.tensor_add(t3[:], t1[:], t2[:])
            nc.sync.dma_start(outp[bt * 128:(bt + 1) * 128], t3[:, 0])

    nc.compile()
    return nc


def _host_prep(x, z, W1, b1, W2, b2):
    oh = np.zeros((B, DC), np.float32)
    oh[np.arange(B)[:, None], np.arange(D)[None, :] * C + x] = 1.0
    # ohs[p, t*256 + b] = oh[b, t*128 + p]
    ohs = np.ascontiguousarray(
        oh.T.reshape(4, 128, B).transpose(1, 0, 2).reshape(128, 4 * B))
    w2s = np.ascontiguousarray(
        W2.reshape(2, 128, DC).transpose(1, 0, 2).reshape(128, 2 * DC))
    b1s = np.ascontiguousarray(b1.reshape(2, 128).T)
    b2s = np.ascontiguousarray(b2.reshape(4, 128).T)
    p_idx = np.arange(128)
    gsel = np.zeros((128, 4 * 32), np.float32)
    for t in range(4):
        gsel[p_idx, t * 32 + t * 8 + p_idx // 16] = 1.0
    ones_s = np.zeros((32, 2), np.float32)
    ones_s[:, 0] = 1.0
    ones_s[28:, 1] = 1.0
    coef = np.zeros((2, 256), np.float32)
    coef[0, 0:128] = -1.0
    coef[1, 128:256] = -1.0
    common = dict(w1=np.ascontiguousarray(W1), w2s=w2s, b1s=b1s, b2s=b2s,
                  ohs=ohs, gsel=gsel, ones_s=ones_s, coef=coef)
    in_maps = []
    for c in range(P):
        m = dict(common)
        m["zT"] = np.ascontiguousarray(z[c * NL:(c + 1) * NL, :].T)
        in_maps.append(m)
    return in_maps


_NC_CACHE = {}


def kernel(x, log_w, z, k, W1, b1, W2, b2, _trace=False, _trace_kwargs=None):
    assert int(k) == K
    x = np.asarray(x, np.int32)
    in_maps = _host_prep(np.asarray(x), np.asarray(z, np.float32),
                         np.asarray(W1, np.float32), np.asarray(b1, np.float32),
                         np.asarray(W2, np.float32), np.asarray(b2, np.float32))
    if "nc" not in _NC_CACHE:
        _NC_CACHE["nc"] = _build_nc()
    nc = _NC_CACHE["nc"]
    res = run_bass_kernel_spmd(
        nc, in_maps, list(range(P)), trace=_trace,
        **(_trace_kwargs or {}))
    out = res.results[0]["out"]
    if _trace:
        _NC_CACHE["last_result"] = res
    return np.asarray(out, np.float32)


# revision 9
# speedup vs baseline: 1.3510x; 1.3510x over previous
"""Trainium2 Bass kernel for nn_CategoricalDecoder (topk_masking).

Math (validated against the reference offline):
  logits = relu(z @ W1 + b1) @ W2 + b2, reshaped (N, D, C)
  logp   = log_softmax(logits, -1)
  scores_full[b,n] = sum_d logp[n, d, x[b,d]]          (all D features)
  scores_tail[b,n] = sum_{d in last 4} logp[n,d,x[b,d]] (the reference's
        (g*mask_full).sum - (g*mask_obs).sum collapses to the last-4 sum)
  top-16 bins by scores_tail; num = scores_full at those bins,
  den = num - scores_tail there; out = lse(num) - lse(den).

Device algorithm (8 cores, bin axis N sharded 1024/core, batch replicated):
  - per-class gather is a matmul with one-hot(x) rows (built on host)
  - log_softmax denominators folded into the score matmul via 2 extra
    contraction rows (coefficients -1) carrying S32[n] = sum_d log(sumexp)
    and S4[n] (last-4 sum); inner softmax needs no max-stabilization
    (|logits| < 5 for this net scale).
  - top-16 per row via DVE max(top-8) + match_replace + max, local per
    shard; AllGather the 8x16 local candidates; the global 16th largest
    is a threshold T; masked logsumexp with mask (tail >= T) needs no
    gather/argmax at all. Partial exp-sums AllReduce'd across cores.
"""

import numpy as np
from contextlib import ExitStack

import concourse.bass as bass
import concourse.bacc as bacc
import concourse.tile as tile
from concourse import mybir
from concourse.bass_utils import run_bass_kernel_spmd

F32 = mybir.dt.float32
F32R = mybir.dt.float32r
AF = mybir.ActivationFunctionType
ALU = mybir.AluOpType

B, N, Lz, H, D, C = 256, 8192, 64, 256, 32, 16
DC = D * C          # 512
P = 8               # cores
NL = N // P         # 1024 bins per core
K = 16
NEG = -1.0e30


def _build_nc():
    nc = bacc.Bacc("TRN2", target_bir_lowering=False, num_devices=P)

    zT = nc.declare_dram_parameter("zT", [Lz, NL], F32, isOutput=False)
    w1 = nc.declare_dram_parameter("w1", [Lz, H], F32, isOutput=False)
    w2s = nc.declare_dram_parameter("w2s", [128, 2 * DC], F32, isOutput=False)
    b1s = nc.declare_dram_parameter("b1s", [128, 2], F32, isOutput=False)
    b2s = nc.declare_dram_parameter("b2s", [128, 4], F32, isOutput=False)
    ohs = nc.declare_dram_parameter("ohs", [128, 4 * 256], F32, isOutput=False)
    gsel = nc.declare_dram_parameter("gsel", [128, 4 * 32], F32, isOutput=False)
    ones_s = nc.declare_dram_parameter("ones_s", [32, 2], F32, isOutput=False)
    coef = nc.declare_dram_parameter("coef", [2, 256], F32, isOutput=False)
    outp = nc.declare_dram_parameter("out", [B], F32, isOutput=True)

    with tile.TileContext(nc) as tc, ExitStack() as ctx:
        const = ctx.enter_context(tc.tile_pool(name="const", bufs=1))
        act = ctx.enter_context(tc.tile_pool(name="act", bufs=1))
        ppool = ctx.enter_context(tc.tile_pool(name="psum", bufs=3, space="PSUM"))
        spsum = ctx.enter_context(tc.tile_pool(name="spsum", bufs=1, space="PSUM"))
        dram = ctx.enter_context(tc.tile_pool(name="dram", bufs=1, space="DRAM"))
        scratch = ctx.enter_context(tc.tile_pool(name="scratch", bufs=2))

        # ---- load params to SBUF
        zT_sb = const.tile([Lz, NL], F32)
        nc.sync.dma_start(zT_sb[:], zT[:])
        w1_sb = const.tile([Lz, H], F32)
        nc.sync.dma_start(w1_sb[:], w1[:])
        w2_sb = const.tile([128, 2 * DC], F32)
        nc.sync.dma_start(w2_sb[:], w2s[:])
        b1_sb = const.tile([128, 2], F32)
        nc.sync.dma_start(b1_sb[:], b1s[:])
        b2_sb = const.tile([128, 4], F32)
        nc.sync.dma_start(b2_sb[:], b2s[:])
        oh_sb = const.tile([128, 4 * 256], F32)
        nc.sync.dma_start(oh_sb[:], ohs[:])
        g_sb = const.tile([128, 4 * 32], F32)
        nc.sync.dma_start(g_sb[:], gsel[:])
        on_sb = const.tile([32, 2], F32)
        nc.sync.dma_start(on_sb[:], ones_s[:])
        cf_sb = const.tile([2, 256], F32)
        nc.sync.dma_start(cf_sb[:], coef[:])

        # ---- stage 1: hT[2][128, NL] = relu(W1.T @ zT + b1)
        hT = []
        for m in range(2):
            ph = ppool.tile([128, NL], F32, tag="mm")
            for f in range(NL // 512):
                nc.tensor.matmul(
                    ph[:, f * 512:(f + 1) * 512],
                    w1_sb[:, m * 128:(m + 1) * 128].bitcast(F32R),
                    zT_sb[:, f * 512:(f + 1) * 512].bitcast(F32R),
                    start=True, stop=True,
                )
            h = act.tile([128, NL], F32, name=f"hT{m}")
            nc.scalar.activation(h[:], ph[:], AF.Relu, bias=b1_sb[:, m:m + 1])
            hT.append(h)

        # ---- stage 2: lin[t] = logitsT + b2 (SBUF), e[t] = exp(lin) (SBUF)
        lin, ee = [], []
        for t in range(4):
            pl = ppool.tile([128, NL], F32, tag="mm")
            for f in range(NL // 512):
                for kk in range(2):
                    nc.tensor.matmul(
                        pl[:, f * 512:(f + 1) * 512],
                        w2_sb[:, kk * DC + t * 128: kk * DC + (t + 1) * 128].bitcast(F32R),
                        hT[kk][:, f * 512:(f + 1) * 512].bitcast(F32R),
                        start=(kk == 0), stop=(kk == 1),
                    )
            li = act.tile([128, NL], F32, name=f"lin{t}")
            nc.vector.tensor_scalar_add(li[:], pl[:], b2_sb[:, t:t + 1])
            ex = act.tile([128, NL], F32, name=f"e{t}")
            nc.scalar.activation(ex[:], pl[:], AF.Exp, bias=b2_sb[:, t:t + 1])
            lin.append(li)
            ee.append(ex)

        # ---- stage 3: group-sums of e -> L = log(sumexp) [32, NL] -> S rows [2, NL]
        pse = spsum.tile([34, NL], F32, tag="se")
        for f in range(NL // 512):
            for t in range(4):
                nc.tensor.matmul(
                    pse[0:32, f * 512:(f + 1) * 512],
                    g_sb[:, t * 32:(t + 1) * 32].bitcast(F32R),
                    ee[t][:, f * 512:(f + 1) * 512].bitcast(F32R),
                    start=(t == 0), stop=(t == 3),
                )
        l_sb = act.tile([32, NL], F32, name="l_sb")
        nc.scalar.activation(l_sb[:], pse[0:32, :], AF.Ln)
        for f in range(NL // 512):
            nc.tensor.matmul(
                pse[32:34, f * 512:(f + 1) * 512], on_sb[:, 0:2].bitcast(F32R),
                l_sb[:, f * 512:(f + 1) * 512].bitcast(F32R),
                start=True, stop=True,
            )
        s_sb = act.tile([2, NL], F32, name="s_sb")
        nc.vector.tensor_copy(s_sb[:], pse[32:34, :])

        # ---- stage 4: score matmuls -> sf/st [bt][128, NL] in SBUF
        sf, st, dn = [], [], []
        for bt in range(2):
            pf = ppool.tile([128, NL], F32, tag="mm")
            pt = ppool.tile([128, NL], F32, tag="mm")
            for f in range(NL // 512):
                sl = slice(f * 512, (f + 1) * 512)
                for t in range(4):
                    nc.tensor.matmul(
                        pf[:, sl],
                        oh_sb[:, t * 256 + bt * 128: t * 256 + (bt + 1) * 128].bitcast(F32R),
                        lin[t][:, sl].bitcast(F32R), start=(t == 0), stop=False,
                    )
                nc.tensor.matmul(
                    pf[:, sl], cf_sb[:, 0:128].bitcast(F32R),
                    s_sb[:, sl].bitcast(F32R),
                    start=False, stop=True,
                )
                nc.tensor.matmul(
                    pt[:, sl],
                    oh_sb[64:128, 3 * 256 + bt * 128: 3 * 256 + (bt + 1) * 128].bitcast(F32R),
                    lin[3][64:128, sl].bitcast(F32R), start=True, stop=False,
                )
                nc.tensor.matmul(
                    pt[:, sl], cf_sb[:, 128:256].bitcast(F32R),
                    s_sb[:, sl].bitcast(F32R),
                    start=False, stop=True,
                )
            f_sb = act.tile([128, NL], F32, name=f"sf{bt}")
            nc.scalar.copy(f_sb[:], pf[:])
            t_sb = act.tile([128, NL], F32, name=f"st{bt}")
            nc.vector.tensor_copy(t_sb[:], pt[:])
            d_sb = act.tile([128, NL], F32, name=f"dn{bt}")
            nc.vector.tensor_sub(d_sb[:], f_sb[:], t_sb[:])
            sf.append(f_sb)
            st.append(t_sb)
            dn.append(d_sb)

        # ---- stage 5: local top-16 of tail + local row maxes; exchange
        xin = dram.tile([B, 18], F32)
        xout = dram.tile([P * B, 18], F32, addr_space="Shared")
        for bt in range(2):
            x_sb = act.tile([128, 18], F32, name=f"x{bt}")
            nc.vector.max(x_sb[:, 0:8], st[bt][:])
            wk = scratch.tile([128, NL], F32, tag="wk")
            nc.vector.match_replace(wk[:], x_sb[:, 0:8], st[bt][:], NEG)
            nc.vector.max(x_sb[:, 8:16], wk[:])
            nc.vector.reduce_max(x_sb[:, 16:17], sf[bt][:], axis=mybir.AxisListType.X)
            nc.vector.reduce_max(x_sb[:, 17:18], dn[bt][:], axis=mybir.AxisListType.X)
            nc.sync.dma_start(xin[bt * 128:(bt + 1) * 128, :], x_sb[:])
        nc.gpsimd.collective_compute(
            "AllGather", ALU.bypass, replica_groups=[list(range(P))],
            ins=[xin[:].opt()], outs=[xout[:].opt()],
        )

        # ---- stage 6: global threshold + masked logsumexp partial sums
        sin = dram.tile([B, 2], F32)
        sout = dram.tile([B, 2], F32, addr_space="Shared")
        gmx = []  # [128,2] per bt: col0 = -gmax_num, col1 = -gmax_den
        tthr = []
        for bt in range(2):
            y_sb = act.tile([128, P, 18], F32, name=f"y{bt}")
            for c in range(P):
                nc.sync.dma_start(
                    y_sb[:, c, :],
                    xout[c * B + bt * 128: c * B + (bt + 1) * 128, :],
                )
            cand = act.tile([128, P * 16], F32, name=f"cand{bt}")
            nc.vector.tensor_copy(
                cand[:].rearrange("p (a b) -> p a b", a=P), y_sb[:, :, 0:16])
            g8 = act.tile([128, 16], F32, name=f"g8{bt}")
            nc.vector.max(g8[:, 0:8], cand[:])
            wk2 = scratch.tile([128, P * 16], F32, tag="wk2")
            nc.vector.match_replace(wk2[:], g8[:, 0:8], cand[:], NEG)
            nc.vector.max(g8[:, 8:16], wk2[:])
            tthr.append(g8)
            gm = act.tile([128, 2], F32, name=f"gm{bt}")
            nc.vector.tensor_reduce(
                gm[:, 0:1], y_sb[:, :, 16:17], axis=mybir.AxisListType.XY,
                op=ALU.max, negate=True)
            nc.vector.tensor_reduce(
                gm[:, 1:2], y_sb[:, :, 17:18], axis=mybir.AxisListType.XY,
                op=ALU.max, negate=True)
            gmx.append(gm)

            exn = scratch.tile([128, NL], F32, tag="exn")
            nc.scalar.activation(exn[:], sf[bt][:], AF.Exp, bias=gm[:, 0:1])
            s2 = act.tile([128, 2], F32, name=f"s2{bt}")
            msk = scratch.tile([128, NL], F32, tag="msk")
            nc.vector.scalar_tensor_tensor(
                msk[:], st[bt][:], g8[:, 15:16], exn[:],
                op0=ALU.is_ge, op1=ALU.mult, accum_out=s2[:, 0:1])
            exd = scratch.tile([128, NL], F32, tag="exn")
            nc.scalar.activation(exd[:], dn[bt][:], AF.Exp, bias=gm[:, 1:2])
            msk2 = scratch.tile([128, NL], F32, tag="msk")
            nc.vector.scalar_tensor_tensor(
                msk2[:], st[bt][:], g8[:, 15:16], exd[:],
                op0=ALU.is_ge, op1=ALU.mult, accum_out=s2[:, 1:2])
            nc.sync.dma_start(sin[bt * 128:(bt + 1) * 128, :], s2[:])

        nc.gpsimd.collective_compute(
            "AllReduce", ALU.add, replica_groups=[list(range(P))],
            ins=[sin[:].opt()], outs=[sout[:].opt()],
        )

        # ---- stage 7: out = (gmax_n + ln(sum_n)) - (gmax_d + ln(sum_d))
        for bt in range(2):
            gs = act.tile([128, 2], F32, name=f"gs{bt}")
            nc.sync.dma_start(gs[:], sout[bt * 128:(bt + 1) * 128, :])
            ln = act.tile([128, 2], F32, name=f"ln{bt}")
            nc.scalar.activation(ln[:], gs[:], AF.Ln)
            # gmx holds NEGATED maxes: out = (ln_n - ln_d) - (negmax_n - negmax_d)
            t1 = act.tile([128, 1], F32, name=f"t1{bt}")
            nc.vector.tensor_sub(t1[:], ln[:, 0:1], ln[:, 1:2])
            t2 = act.tile([128, 1], F32, name=f"t2{bt}")
            nc.vector.tensor_sub(t2[:], gmx[bt][:, 1:2], gmx[bt][:, 0:1])
            t3 = act.tile([128, 1], F32, name=f"t3{bt}")
            nc.vector.tensor_add(t3[:], t1[:], t2[:])
            nc.sync.dma_start(outp[bt * 128:(bt + 1) * 128], t3[:, 0])

    nc.compile()
    return nc


def _host_prep(x, z, W1, b1, W2, b2):
    oh = np.zeros((B, DC), np.float32)
    oh[np.arange(B)[:, None], np.arange(D)[None, :] * C + x] = 1.0
    # ohs[p, t*256 + b] = oh[b, t*128 + p]
    ohs = np.ascontiguousarray(
        oh.T.reshape(4, 128, B).transpose(1, 0, 2).reshape(128, 4 * B))
    w2s = np.ascontiguousarray(
        W2.reshape(2, 128, DC).transpose(1, 0, 2).reshape(128, 2 * DC))
    b1s = np.ascontiguousarray(b1.reshape(2, 128).T)
    b2s = np.ascontiguousarray(b2.reshape(4, 128).T)
    p_idx = np.arange(128)
    gsel = np.zeros((128, 4 * 32), np.float32)
    for t in range(4):
        gsel[p_idx, t * 32 + t * 8 + p_idx // 16] = 1.0
    ones_s = np.zeros((32, 2), np.float32)
    ones_s[:, 0] = 1.0
    ones_s[28:, 1] = 1.0
    coef = np.zeros((2, 256), np.float32)
    coef[0, 0:128] = -1.0
    coef[1, 128:256] = -1.0
    common = dict(w1=np.ascontiguousarray(W1), w2s=w2s, b1s=b1s, b2s=b2s,
                  ohs=ohs, gsel=gsel, ones_s=ones_s, coef=coef)
    in_maps = []
    for c in range(P):
        m = dict(common)
        m["zT"] = np.ascontiguousarray(z[c * NL:(c + 1) * NL, :].T)
        in_maps.append(m)
    return in_maps


_NC_CACHE = {}


def kernel(x, log_w, z, k, W1, b1, W2, b2, _trace=False, _trace_kwargs=None):
    assert int(k) == K
    x = np.asarray(x, np.int32)
    in_maps = _host_prep(np.asarray(x), np.asarray(z, np.float32),
                         np.asarray(W1, np.float32), np.asarray(b1, np.float32),
                         np.asarray(W2, np.float32), np.asarray(b2, np.float32))
    if "nc" not in _NC_CACHE:
        _NC_CACHE["nc"] = _build_nc()
    nc = _NC_CACHE["nc"]
    res = run_bass_kernel_spmd(
        nc, in_maps, list(range(P)), trace=_trace,
        **(_trace_kwargs or {}))
    out = res.results[0]["out"]
    if _trace:
        _NC_CACHE["last_result"] = res
    return np.asarray(out, np.float32)
